# revision 1
# baseline (speedup 1.0000x reference)
"""GCN link predictor on 8 Trainium2 NeuronCores (Bass/Tile) — v2.

Single-fp16 rework of the baseline (rel-err budget is 2e-2; the hi/lo
double-fp16 scheme was ~1e-6):
  - Tables G1 [NPAD, 128] f16 (256B rows), G2 [NPAD, 128] f16 (g2 in cols
    0:64, junk elsewhere), Z [NPAD, 64] f32.
  - One eq build + one matmul per 128-edge chunk (was 2 eq + 3 matmuls).
    eq built on DVE (is_equal+mult tensor_scalar); every ACT_EQ_MOD-th chunk
    built on the idle ACT engine via eq = Relu(val - val*|iota - dest|).
  - All per-chunk index/dest/val metadata preloaded into SBUF once (shared
    by both layers) — removes ~800 small dma_starts.
  - Gather groups of GMAX=32 chunks (4096 rows) per SWDGE instruction.
  - Phase A in fp16 with [128,2,512] wide loads and batched stores.
  - Epilogues on ACT: relu, +b2 via Identity(bias=b2), psum->sbuf copies.
"""
import sys
sys.path.insert(0, '/opt/trn_rl_repo')
import numpy as np
import concourse.bass as bass
import concourse.bacc as bacc
import concourse.mybir as mybir
import concourse.tile as tile
from concourse.bass_utils import run_bass_kernel_spmd
from concourse.masks import make_identity

N = 100000
IN_F, HID, OUT = 256, 128, 64
NNZ = 1600000
EQ = 200000

NCORES = 8
SHR = 12500
SH = 12800
NPAD = SH * NCORES
SEG_R = 25600
NSEG = 4
WIN = 64
WPS = 8
ST = WIN * WPS
NST = SH // ST
NW = SH // WIN
P = 128
EQC = EQ // NCORES

FP = mybir.dt.float32
F16 = mybir.dt.float16
I16 = mybir.dt.int16
GMAX = 8
DSCRATCH = 131072
ACT_EQ_MOD = 0      # if >0: chunks with ci % MOD == MOD-1 build eq on ACT


def _wrap_idx(flat):
    n = flat.shape[0]
    w = flat.reshape(n // 16, 16).T
    return np.tile(w, (8, 1)).copy()


def _preprocess(inputs):
    x = np.asarray(inputs["x"], dtype=np.float32)
    adj_rows = np.asarray(inputs["adj_rows"], dtype=np.int64)
    adj_cols = np.asarray(inputs["adj_cols"], dtype=np.int64)
    adj_vals = np.asarray(inputs["adj_vals"], dtype=np.float32)
    edge_index = np.asarray(inputs["edge_index"], dtype=np.int64)
    W1 = np.asarray(inputs["W1"], dtype=np.float32)
    b1 = np.asarray(inputs["b1"], dtype=np.float32)
    W2 = np.asarray(inputs["W2"], dtype=np.float32)
    b2 = np.asarray(inputs["b2"], dtype=np.float32)

    # ---- per-core edge partition (by destination shard) ----
    bounds = np.searchsorted(adj_rows, np.arange(NCORES + 1) * SHR)
    cores = []
    counts = np.zeros((NCORES, NW * NSEG), dtype=np.int64)
    for r in range(NCORES):
        b0, b1e = bounds[r], bounds[r + 1]
        d_loc = adj_rows[b0:b1e] - r * SHR
        cols = adj_cols[b0:b1e]
        vals = adj_vals[b0:b1e]
        pid = (cols // SHR) * SH + (cols % SHR)
        seg = pid // SEG_R
        sidx = (pid % SEG_R).astype(np.int64)
        win = d_loc // WIN
        key = win * NSEG + seg
        order = np.lexsort((sidx, key))
        cores.append((d_loc[order], sidx[order], vals[order], key[order]))
        counts[r] = np.bincount(key, minlength=NW * NSEG)

    quota = -(-counts.max(axis=0) // P)
    qw = quota.reshape(NW, NSEG)
    for w in range(NW):
        if qw[w].sum() == 0:
            qw[w, 0] = 1

    chunk_win = []
    chunk_seg = []
    st_seg_nchunks = np.zeros((NST, NSEG), dtype=np.int64)
    st_first = {}
    st_last = {}
    maxq_all = int(qw.max())
    cmap = np.full((NW, NSEG, maxq_all), -1, dtype=np.int64)
    for st in range(NST):
        for s in range(NSEG):
            wl = list(range(st * WPS, (st + 1) * WPS))
            mq = max(int(qw[w, s]) for w in wl)
            for k in range(mq):
                for w in wl:
                    if qw[w, s] > k:
                        ci = len(chunk_win)
                        cmap[w, s, k] = ci
                        chunk_win.append(w)
                        chunk_seg.append(s)
                        if st not in st_first:
                            st_first[st] = ci
                        st_last[st] = ci
                        st_seg_nchunks[st, s] += 1
    NCH = len(chunk_win)
    chunk_win = np.array(chunk_win)
    chunk_seg = np.array(chunk_seg)
    chunk_start = np.zeros(NCH, dtype=bool)
    chunk_stop = np.zeros(NCH, dtype=bool)
    for st, ci in st_first.items():
        chunk_start[ci] = True
    for st, ci in st_last.items():
        chunk_stop[ci] = True


    cols16_all = np.zeros((NCORES, NCH, P), dtype=np.int16)
    dest_all = np.zeros((NCORES, NCH, P), dtype=np.float32)
    val_all = np.zeros((NCORES, NCH, P), dtype=np.float32)
    for r in range(NCORES):
        d_loc, sidx, vals, key = cores[r]
        ne = d_loc.shape[0]
        if ne == 0:
            continue
        cnt = counts[r]
        starts = np.zeros(NW * NSEG + 1, dtype=np.int64)
        np.cumsum(cnt, out=starts[1:])
        rank = np.arange(ne) - starts[key]
        w_arr = key // NSEG
        chunk_id = cmap[w_arr, key % NSEG, rank // P]
        slot = rank % P
        cols16_all[r, chunk_id, slot] = sidx.astype(np.int16)
        dest_all[r, chunk_id, slot] = (d_loc - w_arr * WIN).astype(np.float32)
        val_all[r, chunk_id, slot] = vals

    # per-(core, chunk) count of real edges (pads fill the chunk tail)
    realc = np.zeros((NCORES, NCH), dtype=np.int64)
    for r in range(NCORES):
        cnt = counts[r]
        for w in range(NW):
            for s in range(NSEG):
                c = int(cnt[w * NSEG + s])
                for k in range(int(qw[w, s])):
                    realc[r, cmap[w, s, k]] = min(max(c - k * P, 0), P)

    # gather-instruction spans (identical for both layers)
    spans = []
    ci = 0
    for st in range(NST):
        for s in range(NSEG):
            na = int(st_seg_nchunks[st, s])
            if na == 0:
                continue
            for g0 in range(0, na, GMAX):
                spans.append((ci + g0, min(GMAX, na - g0)))
            ci += na
    NSPAN = len(spans)

    # trailing pad slots -> idx -1 (skipped by SWDGE); per-span valid counts
    spancnt = np.zeros((NCORES, NSPAN), dtype=np.int32)
    for r in range(NCORES):
        for j, (c0, nch) in enumerate(spans):
            nvalid = nch * P
            for k in range(nch - 1, -1, -1):
                rc = int(realc[r, c0 + k])
                if rc == 0:
                    if k > 0:
                        cols16_all[r, c0 + k, :] = -1
                        nvalid -= P
                    else:
                        cols16_all[r, c0, 1:] = -1
                        nvalid -= (P - 1)
                        break
                else:
                    if rc < P:
                        cols16_all[r, c0 + k, rc:] = -1
                        nvalid -= (P - rc)
                    break
            spancnt[r, j] = nvalid

    idx_wrapped = np.zeros((NCORES, P, NCH * P // 16), dtype=np.int16)
    for r in range(NCORES):
        idx_wrapped[r] = _wrap_idx(cols16_all[r].reshape(NCH * P))

    # destval32 [P, 2*NCH] f32 (DVE is_equal path): dest | val
    # destval16 [P, 3*NCH] f16 (ACT abs/relu path): -dest | val | -val
    destT = dest_all.transpose(0, 2, 1)
    valT = val_all.transpose(0, 2, 1)
    destval32 = np.concatenate([destT, valT], axis=2).astype(np.float32)
    destval16 = np.concatenate([-destT, valT, -valT], axis=2).astype(np.float16)

    # ---- decode preprocessing ----
    src = edge_index[0]
    dst = edge_index[1]
    spid = (src // SHR) * SH + (src % SHR)
    dpid = (dst // SHR) * SH + (dst % SHR)
    dec = []
    dcounts = np.zeros((NCORES, NSEG * NSEG), dtype=np.int64)
    for r in range(NCORES):
        q0, q1 = r * EQC, (r + 1) * EQC
        ss = spid[q0:q1] // SEG_R
        ds = dpid[q0:q1] // SEG_R
        gkey = ss * NSEG + ds
        order = np.lexsort((np.arange(EQC), gkey))
        dec.append((spid[q0:q1] % SEG_R, dpid[q0:q1] % SEG_R, gkey, order))
        dcounts[r] = np.bincount(gkey, minlength=NSEG * NSEG)
    dquota = -(-dcounts.max(axis=0) // P)
    NQCH = int(dquota.sum())
    dbase = np.concatenate([[0], np.cumsum(dquota)])[:-1]

    sidx_dec = np.zeros((NCORES, NQCH, P), dtype=np.int16)
    didx_dec = np.zeros((NCORES, NQCH, P), dtype=np.int16)
    perm_dec = np.full((NCORES, NQCH, P), -1, dtype=np.int64)
    for r in range(NCORES):
        sloc, dloc, gkey, order = dec[r]
        gk = gkey[order]
        cnt = dcounts[r]
        starts = np.zeros(NSEG * NSEG + 1, dtype=np.int64)
        np.cumsum(cnt, out=starts[1:])
        rank = np.arange(EQC) - starts[gk]
        chunk_id = dbase[gk] + rank // P
        slot = rank % P
        sidx_dec[r, chunk_id, slot] = sloc[order].astype(np.int16)
        didx_dec[r, chunk_id, slot] = dloc[order].astype(np.int16)
        perm_dec[r, chunk_id, slot] = order
    sidx_wr = np.zeros((NCORES, P, NQCH * P // 16), dtype=np.int16)
    didx_wr = np.zeros((NCORES, P, NQCH * P // 16), dtype=np.int16)
    for r in range(NCORES):
        sidx_wr[r] = _wrap_idx(sidx_dec[r].reshape(NQCH * P))
        didx_wr[r] = _wrap_idx(didx_dec[r].reshape(NQCH * P))

    dq = dquota.reshape(NSEG, NSEG)
    src_runs = []
    dst_runs = []
    pos = 0
    for ss in range(NSEG):
        n = int(dq[ss].sum())
        if n:
            src_runs.append((pos, n, ss))
        p2 = pos
        for ds in range(NSEG):
            if dq[ss, ds]:
                dst_runs.append((p2, int(dq[ss, ds]), ds))
            p2 += int(dq[ss, ds])
        pos += n

    # ---- dense inputs ----
    xt = x.T.astype(np.float16)  # [256, N]
    iota64 = np.broadcast_to(np.arange(WIN, dtype=np.float16), (P, WIN)).copy()
    b1t = np.broadcast_to(b1, (P, HID)).astype(np.float32).copy()
    W1s = np.stack([W1[0:P, :], W1[P:IN_F, :]], axis=1).astype(np.float16)
    in_maps = []
    for r in range(NCORES):
        x2 = np.zeros((P, 2, SH), dtype=np.float16)
        x2[:, 0, :SHR] = xt[0:P, r * SHR:(r + 1) * SHR]
        x2[:, 1, :SHR] = xt[P:IN_F, r * SHR:(r + 1) * SHR]
        in_maps.append({
            "x2": x2,
            "W1s": W1s.copy(),
            "b1t": b1t.copy(),
            "w2t": W2.astype(np.float16).copy(),
            "b2c": b2.reshape(OUT, 1).astype(np.float32).copy(),
            "iota": iota64.copy(),
            "colsw": idx_wrapped[r],
            "destval": destval32[r].copy(),
            "destva16": destval16[r].copy(),
            "qsrcw": sidx_wr[r],
            "qdstw": didx_wr[r],
            "spancnt": np.ascontiguousarray(spancnt[r].reshape(1, NSPAN)),
        })

    meta = {
        "NCH": NCH,
        "NSPAN": NSPAN,
        "chunk_win": chunk_win,
        "chunk_seg": chunk_seg,
        "chunk_start": chunk_start,
        "chunk_stop": chunk_stop,
        "st_seg_nchunks": st_seg_nchunks,
        "NQCH": NQCH,
        "src_runs": src_runs,
        "dst_runs": dst_runs,
        "perm_dec": perm_dec,
    }
    return in_maps, meta


def _build(meta, single=False, upto='full', repeat=1):
    NCH = meta["NCH"]
    chunk_win = meta["chunk_win"]
    chunk_start = meta["chunk_start"]
    chunk_stop = meta["chunk_stop"]
    ssn = meta["st_seg_nchunks"]
    NQCH = meta["NQCH"]
    NSPAN = meta["NSPAN"]
    src_runs = meta["src_runs"]
    dst_runs = meta["dst_runs"]

    ncore = 1 if single else NCORES
    nc = bacc.Bacc("TRN2", target_bir_lowering=False, debug=False,
                   num_devices=ncore, dynamic_dma_scratch_size=DSCRATCH,
                   num_swdge_queues=4)
    qrr = [0]

    def _next_q():
        qrr[0] = (qrr[0] + 1) % 4
        return qrr[0]

    def _collective(in_ap, out_handle, rows):
        if single:
            nc.sync.dma_start(out=out_handle[0:rows, :].opt(), in_=in_ap.opt())
        else:
            nc.gpsimd.collective_compute(
                "AllGather", mybir.AluOpType.bypass, replica_groups=rg,
                ins=[in_ap], outs=[out_handle[:]])

    t_x2 = nc.dram_tensor("x2", [P, 2, SH], F16, kind="ExternalInput")
    t_W1s = nc.dram_tensor("W1s", [P, 2, HID], F16, kind="ExternalInput")
    t_b1t = nc.dram_tensor("b1t", [P, HID], FP, kind="ExternalInput")
    t_w2t = nc.dram_tensor("w2t", [HID, OUT], F16, kind="ExternalInput")
    t_b2c = nc.dram_tensor("b2c", [OUT, 1], FP, kind="ExternalInput")
    t_iota = nc.dram_tensor("iota", [P, WIN], F16, kind="ExternalInput")
    t_cols = nc.dram_tensor("colsw", [P, NCH * P // 16], I16, kind="ExternalInput")
    t_dv = nc.dram_tensor("destval", [P, 2 * NCH], FP, kind="ExternalInput")
    t_dva = nc.dram_tensor("destva16", [P, 3 * NCH], F16, kind="ExternalInput")
    t_qs = nc.dram_tensor("qsrcw", [P, NQCH * P // 16], I16, kind="ExternalInput")
    t_qd = nc.dram_tensor("qdstw", [P, NQCH * P // 16], I16, kind="ExternalInput")
    t_cnt = nc.dram_tensor("spancnt", [1, NSPAN], mybir.dt.int32, kind="ExternalInput")

    o_dec = nc.dram_tensor("out", [P, NQCH], FP, kind="ExternalOutput")

    g1_local = nc.dram_tensor("g1_local", [NST, 4, P, HID], F16)
    G1 = nc.dram_tensor("G1full", [NPAD, HID], F16, addr_space="Shared")
    g2_local = nc.dram_tensor("g2_local", [NST, 4, P, 2 * OUT], F16)
    G2 = nc.dram_tensor("G2full", [NPAD, 2 * OUT], F16, addr_space="Shared")
    z_local = nc.dram_tensor("z_local", [NST, 4, P, OUT], FP)
    Z = nc.dram_tensor("Zfull", [NPAD, OUT], FP, addr_space="Shared")

    rg = [list(range(NCORES))]
    Act = mybir.ActivationFunctionType
    cnt_regs = [nc.alloc_register(mybir.EngineType.Pool, f"cntr{i}")
                for i in range(16)]

    with tile.TileContext(nc) as tc:
      for _rep in range(repeat):
            with tc.tile_pool(name="const", bufs=1) as cp:
                w1ab = cp.tile([P, 2, HID], F16)
                nc.sync.dma_start(out=w1ab[:], in_=t_W1s[:])
                b1tt = cp.tile([P, HID], FP)
                nc.sync.dma_start(out=b1tt[:], in_=t_b1t[:])
                w2tt = cp.tile([HID, OUT], F16)
                nc.sync.dma_start(out=w2tt[:], in_=t_w2t[:])
                b2ct = cp.tile([OUT, 1], FP)
                nc.sync.dma_start(out=b2ct[:], in_=t_b2c[:])
                iota_t = cp.tile([P, WIN], F16)
                nc.sync.dma_start(out=iota_t[:], in_=t_iota[:])
                id64 = cp.tile([WIN, WIN], FP)
                make_identity(nc, id64[:])
                # whole-layer metadata preloads (shared by both agg layers)
                colst = cp.tile([P, NCH * P // 16], I16)
                nc.sync.dma_start(out=colst[:], in_=t_cols[:])
                dvt = cp.tile([P, 2 * NCH], FP)
                nc.sync.dma_start(out=dvt[:], in_=t_dv[:])
                if ACT_EQ_MOD:
                    dva = cp.tile([P, 3 * NCH], F16)
                    nc.sync.dma_start(out=dva[:], in_=t_dva[:])
                qst = cp.tile([P, NQCH * P // 16], I16)
                nc.sync.dma_start(out=qst[:], in_=t_qs[:])
                qdt = cp.tile([P, NQCH * P // 16], I16)
                nc.sync.dma_start(out=qdt[:], in_=t_qd[:])
                cntt = cp.tile([1, NSPAN], mybir.dt.int32)
                nc.sync.dma_start(out=cntt[:], in_=t_cnt[:])

                # ================= Phase A: G1 = x @ W1 + b1 =================
                with nc.named_scope("phaseA"):
                    with (tc.tile_pool(name="xa", bufs=3) as xa,
                          tc.tile_pool(name="stA", bufs=3) as stA,
                          tc.tile_pool(name="psA", bufs=3, space="PSUM") as psA):
                        for t in range(NST):
                            xt_ = xa.tile([P, 2, ST], F16, tag="x2")
                            nc.sync.dma_start(
                                out=xt_[:], in_=t_x2[:, :, t * ST:(t + 1) * ST])
                            stageA = stA.tile([P, 4, HID], F16, tag="stA")
                            for j in range(4):
                                pa = psA.tile([P, HID], FP, space="PSUM", tag="pa")
                                nc.tensor.matmul(
                                    out=pa[:], lhsT=xt_[:, 0, j * P:(j + 1) * P],
                                    rhs=w1ab[:, 0, :], start=True, stop=False)
                                nc.tensor.matmul(
                                    out=pa[:], lhsT=xt_[:, 1, j * P:(j + 1) * P],
                                    rhs=w1ab[:, 1, :], start=False, stop=True)
                                nc.vector.tensor_tensor(
                                    out=stageA[:, j, :], in0=pa[:], in1=b1tt[:],
                                    op=mybir.AluOpType.add)
                            nc.sync.dma_start(
                                out=g1_local[t].transpose([1, 0, 2]), in_=stageA[:])
                    _collective(g1_local[:], G1, SH)

                # ============ Phase B / C: aggregation layers ============
                def agg_layer(layer, table, feat, out_local, do_g2):
                    scope = f"agg{layer}"
                    with nc.named_scope(scope):
                        with (tc.tile_pool(name=f"gm{layer}", bufs=3) as gm,
                              tc.tile_pool(name=f"eq{layer}", bufs=16) as eqp,
                              tc.tile_pool(name=f"tm{layer}", bufs=4) as tmp,
                              tc.tile_pool(name=f"ep{layer}", bufs=3) as ep,
                              tc.tile_pool(name=f"ps{layer}", bufs=2, space="PSUM") as psp,
                              tc.tile_pool(name=f"pg{layer}", bufs=2, space="PSUM") as pgp,
                              tc.tile_pool(name=f"pt{layer}", bufs=2, space="PSUM") as ptp):
                            ci = 0
                            js = 0
                            for st in range(NST):
                                pst = psp.tile([feat, ST], FP, space="PSUM", tag="agg")
                                for s in range(NSEG):
                                    nch_all = int(ssn[st, s])
                                    if nch_all == 0:
                                        continue
                                    ci0 = ci
                                    for g0 in range(0, nch_all, GMAX):
                                        nch = min(GMAX, nch_all - g0)
                                        gt = gm.tile([P, nch, P], F16, tag="msgs")
                                        nval = cnt_regs[js % 16]
                                        nc.gpsimd.reg_load(
                                            nval, cntt[0:1, js:js + 1])
                                        js += 1
                                        nc.gpsimd.dma_gather(
                                            out_ap=gt[:],
                                            in_ap=table[s * SEG_R:(s + 1) * SEG_R, :],
                                            idxs_ap=colst[:, (ci0 + g0) * P // 16:
                                                          (ci0 + g0 + nch) * P // 16],
                                            num_idxs=nch * P, num_idxs_reg=nval,
                                            elem_size=P, queue_num=_next_q())
                                        for k in range(nch):
                                            c = ci0 + g0 + k
                                            wc = (int(chunk_win[c]) % WPS) * WIN
                                            eq = eqp.tile([P, WIN], F16, tag="eq")
                                            if ACT_EQ_MOD and c % ACT_EQ_MOD == ACT_EQ_MOD - 1:
                                                ab = tmp.tile([P, WIN], F16, tag="ab")
                                                nc.scalar.activation(
                                                    out=ab[:], in_=iota_t[:],
                                                    func=Act.Abs,
                                                    bias=dva[:, c:c + 1])
                                                nc.scalar.activation(
                                                    out=eq[:], in_=ab[:],
                                                    func=Act.Relu,
                                                    bias=dva[:, NCH + c:NCH + c + 1],
                                                    scale=dva[:, 2 * NCH + c:2 * NCH + c + 1])
                                            else:
                                                nc.vector.tensor_scalar(
                                                    out=eq[:], in0=iota_t[:],
                                                    scalar1=dvt[:, c:c + 1],
                                                    scalar2=dvt[:, NCH + c:NCH + c + 1],
                                                    op0=mybir.AluOpType.is_equal,
                                                    op1=mybir.AluOpType.mult)
                                            nc.tensor.matmul(
                                                out=pst[:, wc:wc + WIN],
                                                lhsT=gt[:, k, 0:feat], rhs=eq[:],
                                                start=bool(chunk_start[c]),
                                                stop=bool(chunk_stop[c]))
                                        ci += nch
                                # epilogue for supertile st
                                if do_g2:
                                    rt = ep.tile([P, ST], F16, tag="r1")
                                    nc.scalar.activation(out=rt[:], in_=pst[:],
                                                         func=Act.Relu)
                                    pg = pgp.tile([OUT, ST], FP, space="PSUM", tag="g2")
                                    nc.tensor.matmul(out=pg[:], lhsT=w2tt[:], rhs=rt[:],
                                                     start=True, stop=True)
                                    sb = ep.tile([OUT, ST], FP, tag="g2sb")
                                    nc.scalar.activation(out=sb[:], in_=pg[:],
                                                         func=Act.Identity,
                                                         bias=b2ct[:])
                                    stage = ep.tile([P, 4, OUT], F16, tag="stg")
                                else:
                                    sb = ep.tile([OUT, ST], FP, tag="zsb")
                                    nc.scalar.activation(out=sb[:], in_=pst[:],
                                                         func=Act.Copy)
                                    stage = ep.tile([P, 4, OUT], FP, tag="stgz")
                                for j in range(4):
                                    tp = ptp.tile([P, OUT], FP, space="PSUM", tag="tp")
                                    nc.tensor.transpose(
                                        out=tp[:], in_=sb[:, j * P:(j + 1) * P],
                                        identity=id64[:])
                                    nc.scalar.activation(out=stage[:, j, :], in_=tp[:],
                                                         func=Act.Copy)
                                if do_g2:
                                    nc.sync.dma_start(
                                        out=out_local[st][:, :, 0:OUT].transpose([1, 0, 2]),
                                        in_=stage[:])
                                else:
                                    nc.sync.dma_start(
                                        out=out_local[st].transpose([1, 0, 2]),
                                        in_=stage[:])

                if upto != 'A':
                    agg_layer(1, G1, HID, g2_local, do_g2=True)
                    with nc.named_scope("ag2"):
                        _collective(g2_local[:].opt(), G2, SH)
                if upto in ('L2', 'full'):
                    agg_layer(2, G2, OUT, z_local, do_g2=False)
                    with nc.named_scope("ag3"):
                        _collective(z_local[:].opt(), Z, SH)

                # ================= Phase D: decode =================
                if upto == 'full':
                 with nc.named_scope("decode"):
                    with (tc.tile_pool(name="qs", bufs=1) as qs,
                          tc.tile_pool(name="qd", bufs=2) as qd,
                          tc.tile_pool(name="qo", bufs=1) as qo):
                        red_all = qo.tile([P, NQCH], FP, tag="redall")

                        def gath(pool, tag, seg, cb, nch, idxt):
                            t = pool.tile([P, nch, OUT], FP, tag=tag)
                            for g0 in range(0, nch, GMAX):
                                n = min(GMAX, nch - g0)
                                nc.gpsimd.dma_gather(
                                    out_ap=t[:, g0:g0 + n, :],
                                    in_ap=Z[seg * SEG_R:(seg + 1) * SEG_R, :],
                                    idxs_ap=idxt[:, (cb + g0) * P // 16:
                                                 (cb + g0 + n) * P // 16],
                                    num_idxs=n * P, num_idxs_reg=n * P,
                                    elem_size=OUT, queue_num=_next_q())
                            return t

                        for (cb, nch, ss) in src_runs:
                            zs = gath(qs, "zs", ss, cb, nch, qst)
                            for (cb2, nch2, ds) in [x for x in dst_runs
                                                    if cb <= x[0] < cb + nch]:
                                zd = gath(qd, "zd", ds, cb2, nch2, qdt)
                                prod = qd.tile([P, nch2, OUT], FP, tag="prod")
                                nc.vector.tensor_tensor(
                                    out=prod[:], in0=zs[:, cb2 - cb:cb2 - cb + nch2, :],
                                    in1=zd[:], op=mybir.AluOpType.mult)
                                nc.vector.tensor_reduce(
                                    out=red_all[:, cb2:cb2 + nch2], in_=prod[:],
                                    axis=mybir.AxisListType.X, op=mybir.AluOpType.add)
                        nc.sync.dma_start(out=o_dec[:], in_=red_all[:])

    nc.compile()
    return nc


_BUILD_CACHE = {}


def _meta_key(meta):
    import hashlib
    h = hashlib.sha256()
    h.update(np.asarray(meta["chunk_win"]).tobytes())
    h.update(np.asarray(meta["chunk_seg"]).tobytes())
    h.update(np.asarray(meta["st_seg_nchunks"]).tobytes())
    h.update(repr((meta["NCH"], meta["NQCH"], meta["src_runs"], meta["dst_runs"])).encode())
    return h.hexdigest()


def kernel(**inputs):
    in_maps, meta = _preprocess(inputs)
    key = _meta_key(meta)
    if key not in _BUILD_CACHE:
        _BUILD_CACHE[key] = _build(meta)
    nc = _BUILD_CACHE[key]
    res = run_bass_kernel_spmd(nc, in_maps, core_ids=list(range(NCORES)))
    kernel.last_results = res

    out = np.zeros(EQ, dtype=np.float32)
    perm_dec = meta["perm_dec"]
    for r in range(NCORES):
        od = res.results[r]["out"]
        pr = perm_dec[r]
        valid = pr >= 0
        out[r * EQC + pr[valid]] = od.T[valid]
    return out



# revision 6
# speedup vs baseline: 1.1654x; 1.1654x over previous
"""GCN link predictor on 8 Trainium2 NeuronCores (Bass/Tile) — v2.

Single-fp16 rework of the baseline (rel-err budget is 2e-2; the hi/lo
double-fp16 scheme was ~1e-6):
  - Tables G1 [NPAD, 128] f16 (256B rows), G2 [NPAD, 128] f16 (g2 in cols
    0:64, junk elsewhere), Z [NPAD, 64] f32.
  - One eq build + one matmul per 128-edge chunk (was 2 eq + 3 matmuls).
    eq built on DVE (is_equal+mult tensor_scalar); every ACT_EQ_MOD-th chunk
    built on the idle ACT engine via eq = Relu(val - val*|iota - dest|).
  - All per-chunk index/dest/val metadata preloaded into SBUF once (shared
    by both layers) — removes ~800 small dma_starts.
  - Gather groups of GMAX=32 chunks (4096 rows) per SWDGE instruction.
  - Phase A in fp16 with [128,2,512] wide loads and batched stores.
  - Epilogues on ACT: relu, +b2 via Identity(bias=b2), psum->sbuf copies.
"""
import sys
sys.path.insert(0, '/opt/trn_rl_repo')
import numpy as np
import concourse.bass as bass
import concourse.bacc as bacc
import concourse.mybir as mybir
import concourse.tile as tile
from concourse.bass_utils import run_bass_kernel_spmd
from concourse.masks import make_identity

N = 100000
IN_F, HID, OUT = 256, 128, 64
NNZ = 1600000
EQ = 200000

NCORES = 8
SHR = 12500
SH = 12800
NPAD = SH * NCORES
SEG_R = 25600
NSEG = 4
WIN = 64
WPS = 8
ST = WIN * WPS
NST = SH // ST
NW = SH // WIN
P = 128
EQC = EQ // NCORES

FP = mybir.dt.float32
F16 = mybir.dt.float16
I16 = mybir.dt.int16
GMAX = 8
DSCRATCH = 131072
ACT_EQ_MOD = 0      # if >0: chunks with ci % MOD == MOD-1 build eq on ACT


def _wrap_idx(flat):
    n = flat.shape[0]
    w = flat.reshape(n // 16, 16).T
    return np.tile(w, (8, 1)).copy()


def _preprocess(inputs):
    x = np.asarray(inputs["x"], dtype=np.float32)
    adj_rows = np.asarray(inputs["adj_rows"], dtype=np.int64)
    adj_cols = np.asarray(inputs["adj_cols"], dtype=np.int64)
    adj_vals = np.asarray(inputs["adj_vals"], dtype=np.float32)
    edge_index = np.asarray(inputs["edge_index"], dtype=np.int64)
    W1 = np.asarray(inputs["W1"], dtype=np.float32)
    b1 = np.asarray(inputs["b1"], dtype=np.float32)
    W2 = np.asarray(inputs["W2"], dtype=np.float32)
    b2 = np.asarray(inputs["b2"], dtype=np.float32)

    # ---- per-core edge partition (by destination shard) ----
    bounds = np.searchsorted(adj_rows, np.arange(NCORES + 1) * SHR)
    cores = []
    counts = np.zeros((NCORES, NW * NSEG), dtype=np.int64)
    for r in range(NCORES):
        b0, b1e = bounds[r], bounds[r + 1]
        d_loc = adj_rows[b0:b1e] - r * SHR
        cols = adj_cols[b0:b1e]
        vals = adj_vals[b0:b1e]
        pid = (cols // SHR) * SH + (cols % SHR)
        seg = pid // SEG_R
        sidx = (pid % SEG_R).astype(np.int64)
        win = d_loc // WIN
        key = win * NSEG + seg
        order = np.lexsort((sidx, key))
        cores.append((d_loc[order], sidx[order], vals[order], key[order]))
        counts[r] = np.bincount(key, minlength=NW * NSEG)

    quota = -(-counts.max(axis=0) // P)
    qw = quota.reshape(NW, NSEG)
    for w in range(NW):
        if qw[w].sum() == 0:
            qw[w, 0] = 1

    chunk_win = []
    chunk_seg = []
    st_seg_nchunks = np.zeros((NST, NSEG), dtype=np.int64)
    st_first = {}
    st_last = {}
    maxq_all = int(qw.max())
    cmap = np.full((NW, NSEG, maxq_all), -1, dtype=np.int64)
    for st in range(NST):
        for s in range(NSEG):
            wl = list(range(st * WPS, (st + 1) * WPS))
            mq = max(int(qw[w, s]) for w in wl)
            for k in range(mq):
                for w in wl:
                    if qw[w, s] > k:
                        ci = len(chunk_win)
                        cmap[w, s, k] = ci
                        chunk_win.append(w)
                        chunk_seg.append(s)
                        if st not in st_first:
                            st_first[st] = ci
                        st_last[st] = ci
                        st_seg_nchunks[st, s] += 1
    NCH = len(chunk_win)
    chunk_win = np.array(chunk_win)
    chunk_seg = np.array(chunk_seg)
    chunk_start = np.zeros(NCH, dtype=bool)
    chunk_stop = np.zeros(NCH, dtype=bool)
    for st, ci in st_first.items():
        chunk_start[ci] = True
    for st, ci in st_last.items():
        chunk_stop[ci] = True


    cols16_all = np.zeros((NCORES, NCH, P), dtype=np.int16)
    dest_all = np.zeros((NCORES, NCH, P), dtype=np.float32)
    val_all = np.zeros((NCORES, NCH, P), dtype=np.float32)
    for r in range(NCORES):
        d_loc, sidx, vals, key = cores[r]
        ne = d_loc.shape[0]
        if ne == 0:
            continue
        cnt = counts[r]
        starts = np.zeros(NW * NSEG + 1, dtype=np.int64)
        np.cumsum(cnt, out=starts[1:])
        rank = np.arange(ne) - starts[key]
        w_arr = key // NSEG
        chunk_id = cmap[w_arr, key % NSEG, rank // P]
        slot = rank % P
        cols16_all[r, chunk_id, slot] = sidx.astype(np.int16)
        dest_all[r, chunk_id, slot] = (d_loc - w_arr * WIN).astype(np.float32)
        val_all[r, chunk_id, slot] = vals

    # per-(core, chunk) count of real edges (pads fill the chunk tail)
    realc = np.zeros((NCORES, NCH), dtype=np.int64)
    for r in range(NCORES):
        cnt = counts[r]
        for w in range(NW):
            for s in range(NSEG):
                c = int(cnt[w * NSEG + s])
                for k in range(int(qw[w, s])):
                    realc[r, cmap[w, s, k]] = min(max(c - k * P, 0), P)

    # gather-instruction spans (identical for both layers)
    spans = []
    ci = 0
    for st in range(NST):
        for s in range(NSEG):
            na = int(st_seg_nchunks[st, s])
            if na == 0:
                continue
            for g0 in range(0, na, GMAX):
                spans.append((ci + g0, min(GMAX, na - g0)))
            ci += na
    NSPAN = len(spans)

    # trailing pad slots -> idx -1 (skipped by SWDGE); per-span valid counts
    spancnt = np.zeros((NCORES, NSPAN), dtype=np.int32)
    for r in range(NCORES):
        for j, (c0, nch) in enumerate(spans):
            nvalid = nch * P
            for k in range(nch - 1, -1, -1):
                rc = int(realc[r, c0 + k])
                if rc == 0:
                    if k > 0:
                        cols16_all[r, c0 + k, :] = -1
                        nvalid -= P
                    else:
                        cols16_all[r, c0, 1:] = -1
                        nvalid -= (P - 1)
                        break
                else:
                    if rc < P:
                        cols16_all[r, c0 + k, rc:] = -1
                        nvalid -= (P - rc)
                    break
            spancnt[r, j] = nvalid

    idx_wrapped = np.zeros((NCORES, P, NCH * P // 16), dtype=np.int16)
    for r in range(NCORES):
        idx_wrapped[r] = _wrap_idx(cols16_all[r].reshape(NCH * P))

    # destval32 [P, 2*NCH] f32 (DVE is_equal path): dest | val
    # destval16 [P, 3*NCH] f16 (ACT abs/relu path): -dest | val | -val
    destT = dest_all.transpose(0, 2, 1)
    valT = val_all.transpose(0, 2, 1)
    destval32 = np.concatenate([destT, valT], axis=2).astype(np.float32)
    destval16 = np.concatenate([-destT, valT, -valT], axis=2).astype(np.float16)

    # ---- decode preprocessing ----
    src = edge_index[0]
    dst = edge_index[1]
    spid = (src // SHR) * SH + (src % SHR)
    dpid = (dst // SHR) * SH + (dst % SHR)
    dec = []
    dcounts = np.zeros((NCORES, NSEG * NSEG), dtype=np.int64)
    for r in range(NCORES):
        q0, q1 = r * EQC, (r + 1) * EQC
        ss = spid[q0:q1] // SEG_R
        ds = dpid[q0:q1] // SEG_R
        gkey = ss * NSEG + ds
        order = np.lexsort((np.arange(EQC), gkey))
        dec.append((spid[q0:q1] % SEG_R, dpid[q0:q1] % SEG_R, gkey, order))
        dcounts[r] = np.bincount(gkey, minlength=NSEG * NSEG)
    dquota = -(-dcounts.max(axis=0) // P)
    NQCH = int(dquota.sum())
    dbase = np.concatenate([[0], np.cumsum(dquota)])[:-1]

    sidx_dec = np.zeros((NCORES, NQCH, P), dtype=np.int16)
    didx_dec = np.zeros((NCORES, NQCH, P), dtype=np.int16)
    perm_dec = np.full((NCORES, NQCH, P), -1, dtype=np.int64)
    for r in range(NCORES):
        sloc, dloc, gkey, order = dec[r]
        gk = gkey[order]
        cnt = dcounts[r]
        starts = np.zeros(NSEG * NSEG + 1, dtype=np.int64)
        np.cumsum(cnt, out=starts[1:])
        rank = np.arange(EQC) - starts[gk]
        chunk_id = dbase[gk] + rank // P
        slot = rank % P
        sidx_dec[r, chunk_id, slot] = sloc[order].astype(np.int16)
        didx_dec[r, chunk_id, slot] = dloc[order].astype(np.int16)
        perm_dec[r, chunk_id, slot] = order
    sidx_wr = np.zeros((NCORES, P, NQCH * P // 16), dtype=np.int16)
    didx_wr = np.zeros((NCORES, P, NQCH * P // 16), dtype=np.int16)
    for r in range(NCORES):
        sidx_wr[r] = _wrap_idx(sidx_dec[r].reshape(NQCH * P))
        didx_wr[r] = _wrap_idx(didx_dec[r].reshape(NQCH * P))

    dq = dquota.reshape(NSEG, NSEG)
    src_runs = []
    dst_runs = []
    pos = 0
    for ss in range(NSEG):
        n = int(dq[ss].sum())
        if n:
            src_runs.append((pos, n, ss))
        p2 = pos
        for ds in range(NSEG):
            if dq[ss, ds]:
                dst_runs.append((p2, int(dq[ss, ds]), ds))
            p2 += int(dq[ss, ds])
        pos += n

    # ---- dense inputs ----
    xt = x.T.astype(np.float16)  # [256, N]
    iota64 = np.broadcast_to(np.arange(WIN, dtype=np.float16), (P, WIN)).copy()
    b1t = np.broadcast_to(b1, (P, HID)).astype(np.float32).copy()
    W1s = np.stack([W1[0:P, :], W1[P:IN_F, :]], axis=1).astype(np.float16)
    in_maps = []
    for r in range(NCORES):
        x2 = np.zeros((P, 2, SH), dtype=np.float16)
        x2[:, 0, :SHR] = xt[0:P, r * SHR:(r + 1) * SHR]
        x2[:, 1, :SHR] = xt[P:IN_F, r * SHR:(r + 1) * SHR]
        in_maps.append({
            "x2": x2,
            "W1s": W1s.copy(),
            "b1t": b1t.copy(),
            "w2t": W2.astype(np.float16).copy(),
            "b2c": b2.reshape(OUT, 1).astype(np.float32).copy(),
            "iota": iota64.copy(),
            "colsw": idx_wrapped[r],
            "destval": destval32[r].copy(),
            "destva16": destval16[r].copy(),
            "qsrcw": sidx_wr[r],
            "qdstw": didx_wr[r],
            "spancnt": np.ascontiguousarray(spancnt[r].reshape(1, NSPAN)),
        })

    meta = {
        "NCH": NCH,
        "NSPAN": NSPAN,
        "chunk_win": chunk_win,
        "chunk_seg": chunk_seg,
        "chunk_start": chunk_start,
        "chunk_stop": chunk_stop,
        "st_seg_nchunks": st_seg_nchunks,
        "NQCH": NQCH,
        "src_runs": src_runs,
        "dst_runs": dst_runs,
        "perm_dec": perm_dec,
    }
    return in_maps, meta


def _build(meta, single=False, upto='full', repeat=1, no_coll=False):
    NCH = meta["NCH"]
    chunk_win = meta["chunk_win"]
    chunk_start = meta["chunk_start"]
    chunk_stop = meta["chunk_stop"]
    ssn = meta["st_seg_nchunks"]
    NQCH = meta["NQCH"]
    NSPAN = meta["NSPAN"]
    src_runs = meta["src_runs"]
    dst_runs = meta["dst_runs"]

    ncore = 1 if single else NCORES
    nc = bacc.Bacc("TRN2", target_bir_lowering=False, debug=False,
                   num_devices=ncore, dynamic_dma_scratch_size=DSCRATCH,
                   num_swdge_queues=4)
    qrr = [0]

    def _next_q():
        qrr[0] = (qrr[0] + 1) % 4
        return qrr[0]

    def _collective(in_ap, out_handle, rows):
        if single or no_coll:
            nc.sync.dma_start(out=out_handle[0:rows, :].opt(), in_=in_ap.opt())
        else:
            nc.gpsimd.collective_compute(
                "AllGather", mybir.AluOpType.bypass, replica_groups=rg,
                ins=[in_ap], outs=[out_handle[:]])

    t_x2 = nc.dram_tensor("x2", [P, 2, SH], F16, kind="ExternalInput")
    t_W1s = nc.dram_tensor("W1s", [P, 2, HID], F16, kind="ExternalInput")
    t_b1t = nc.dram_tensor("b1t", [P, HID], FP, kind="ExternalInput")
    t_w2t = nc.dram_tensor("w2t", [HID, OUT], F16, kind="ExternalInput")
    t_b2c = nc.dram_tensor("b2c", [OUT, 1], FP, kind="ExternalInput")
    t_iota = nc.dram_tensor("iota", [P, WIN], F16, kind="ExternalInput")
    t_cols = nc.dram_tensor("colsw", [P, NCH * P // 16], I16, kind="ExternalInput")
    t_dv = nc.dram_tensor("destval", [P, 2 * NCH], FP, kind="ExternalInput")
    t_dva = nc.dram_tensor("destva16", [P, 3 * NCH], F16, kind="ExternalInput")
    t_qs = nc.dram_tensor("qsrcw", [P, NQCH * P // 16], I16, kind="ExternalInput")
    t_qd = nc.dram_tensor("qdstw", [P, NQCH * P // 16], I16, kind="ExternalInput")
    t_cnt = nc.dram_tensor("spancnt", [1, NSPAN], mybir.dt.int32, kind="ExternalInput")

    o_dec = nc.dram_tensor("out", [P, NQCH], FP, kind="ExternalOutput")

    g1_local = nc.dram_tensor("g1_local", [NST, 4, P, HID], F16)
    G1 = nc.dram_tensor("G1full", [NPAD, HID], F16, addr_space="Shared")
    g2_local = nc.dram_tensor("g2_local", [NST, 4, P, 2 * OUT], F16)
    G2 = nc.dram_tensor("G2full", [NPAD, 2 * OUT], F16, addr_space="Shared")
    z_local = nc.dram_tensor("z_local", [NST, 4, P, OUT], FP)
    Z = nc.dram_tensor("Zfull", [NPAD, OUT], FP, addr_space="Shared")

    rg = [list(range(NCORES))]
    Act = mybir.ActivationFunctionType
    cnt_regs = [nc.alloc_register(mybir.EngineType.Pool, f"cntr{i}")
                for i in range(16)]

    with tile.TileContext(nc) as tc:
      for _rep in range(repeat):
            with tc.tile_pool(name="const", bufs=1) as cp:
                w1ab = cp.tile([P, 2, HID], F16)
                nc.sync.dma_start(out=w1ab[:], in_=t_W1s[:])
                b1tt = cp.tile([P, HID], FP)
                nc.sync.dma_start(out=b1tt[:], in_=t_b1t[:])
                w2tt = cp.tile([HID, OUT], F16)
                nc.sync.dma_start(out=w2tt[:], in_=t_w2t[:])
                b2ct = cp.tile([OUT, 1], FP)
                nc.sync.dma_start(out=b2ct[:], in_=t_b2c[:])
                iota_t = cp.tile([P, WIN], F16)
                nc.sync.dma_start(out=iota_t[:], in_=t_iota[:])
                id64 = cp.tile([WIN, WIN], FP)
                make_identity(nc, id64[:])
                # whole-layer metadata preloads (shared by both agg layers)
                colst = cp.tile([P, NCH * P // 16], I16)
                nc.sync.dma_start(out=colst[:], in_=t_cols[:])
                dvt = cp.tile([P, 2 * NCH], FP)
                nc.sync.dma_start(out=dvt[:], in_=t_dv[:])
                if ACT_EQ_MOD:
                    dva = cp.tile([P, 3 * NCH], F16)
                    nc.sync.dma_start(out=dva[:], in_=t_dva[:])
                qst = cp.tile([P, NQCH * P // 16], I16)
                nc.sync.dma_start(out=qst[:], in_=t_qs[:])
                qdt = cp.tile([P, NQCH * P // 16], I16)
                nc.sync.dma_start(out=qdt[:], in_=t_qd[:])
                cntt = cp.tile([1, NSPAN], mybir.dt.int32)
                nc.sync.dma_start(out=cntt[:], in_=t_cnt[:])

                # ================= Phase A: G1 = x @ W1 + b1 =================
                with nc.named_scope("phaseA"):
                    with (tc.tile_pool(name="xa", bufs=3) as xa,
                          tc.tile_pool(name="stA", bufs=3) as stA,
                          tc.tile_pool(name="psA", bufs=3, space="PSUM") as psA):
                        for t in range(NST):
                            xt_ = xa.tile([P, 2, ST], F16, tag="x2")
                            nc.sync.dma_start(
                                out=xt_[:], in_=t_x2[:, :, t * ST:(t + 1) * ST])
                            stageA = stA.tile([P, 4, HID], F16, tag="stA")
                            for j in range(4):
                                pa = psA.tile([P, HID], FP, space="PSUM", tag="pa")
                                nc.tensor.matmul(
                                    out=pa[:], lhsT=xt_[:, 0, j * P:(j + 1) * P],
                                    rhs=w1ab[:, 0, :], start=True, stop=False)
                                nc.tensor.matmul(
                                    out=pa[:], lhsT=xt_[:, 1, j * P:(j + 1) * P],
                                    rhs=w1ab[:, 1, :], start=False, stop=True)
                                nc.vector.tensor_tensor(
                                    out=stageA[:, j, :], in0=pa[:], in1=b1tt[:],
                                    op=mybir.AluOpType.add)
                            nc.sync.dma_start(
                                out=g1_local[t].transpose([1, 0, 2]), in_=stageA[:])
                    _collective(g1_local[:], G1, SH)

                # ============ Phase B / C: aggregation layers ============
                def agg_layer(layer, table, feat, out_local, do_g2):
                    scope = f"agg{layer}"
                    with nc.named_scope(scope):
                        with (tc.tile_pool(name=f"gm{layer}", bufs=3) as gm,
                              tc.tile_pool(name=f"eq{layer}", bufs=16) as eqp,
                              tc.tile_pool(name=f"tm{layer}", bufs=4) as tmp,
                              tc.tile_pool(name=f"ep{layer}", bufs=3) as ep,
                              tc.tile_pool(name=f"ps{layer}", bufs=2, space="PSUM") as psp,
                              tc.tile_pool(name=f"pg{layer}", bufs=2, space="PSUM") as pgp,
                              tc.tile_pool(name=f"pt{layer}", bufs=2, space="PSUM") as ptp):
                            ci = 0
                            js = 0
                            for st in range(NST):
                                pst = psp.tile([feat, ST], FP, space="PSUM", tag="agg")
                                for s in range(NSEG):
                                    nch_all = int(ssn[st, s])
                                    if nch_all == 0:
                                        continue
                                    ci0 = ci
                                    for g0 in range(0, nch_all, GMAX):
                                        nch = min(GMAX, nch_all - g0)
                                        gt = gm.tile([P, nch, P], F16, tag="msgs")
                                        nval = cnt_regs[js % 16]
                                        nc.gpsimd.reg_load(
                                            nval, cntt[0:1, js:js + 1])
                                        js += 1
                                        nc.gpsimd.dma_gather(
                                            out_ap=gt[:],
                                            in_ap=table[s * SEG_R:(s + 1) * SEG_R, :],
                                            idxs_ap=colst[:, (ci0 + g0) * P // 16:
                                                          (ci0 + g0 + nch) * P // 16],
                                            num_idxs=nch * P, num_idxs_reg=nval,
                                            elem_size=P, queue_num=_next_q())
                                        for k in range(nch):
                                            c = ci0 + g0 + k
                                            wc = (int(chunk_win[c]) % WPS) * WIN
                                            eq = eqp.tile([P, WIN], F16, tag="eq")
                                            if ACT_EQ_MOD and c % ACT_EQ_MOD == ACT_EQ_MOD - 1:
                                                ab = tmp.tile([P, WIN], F16, tag="ab")
                                                nc.scalar.activation(
                                                    out=ab[:], in_=iota_t[:],
                                                    func=Act.Abs,
                                                    bias=dva[:, c:c + 1])
                                                nc.scalar.activation(
                                                    out=eq[:], in_=ab[:],
                                                    func=Act.Relu,
                                                    bias=dva[:, NCH + c:NCH + c + 1],
                                                    scale=dva[:, 2 * NCH + c:2 * NCH + c + 1])
                                            else:
                                                nc.vector.tensor_scalar(
                                                    out=eq[:], in0=iota_t[:],
                                                    scalar1=dvt[:, c:c + 1],
                                                    scalar2=dvt[:, NCH + c:NCH + c + 1],
                                                    op0=mybir.AluOpType.is_equal,
                                                    op1=mybir.AluOpType.mult)
                                            nc.tensor.matmul(
                                                out=pst[:, wc:wc + WIN],
                                                lhsT=gt[:, k, 0:feat], rhs=eq[:],
                                                start=bool(chunk_start[c]),
                                                stop=bool(chunk_stop[c]))
                                        ci += nch
                                # epilogue for supertile st
                                if do_g2:
                                    rt = ep.tile([P, ST], F16, tag="r1")
                                    nc.scalar.activation(out=rt[:], in_=pst[:],
                                                         func=Act.Relu)
                                    pg = pgp.tile([OUT, ST], FP, space="PSUM", tag="g2")
                                    nc.tensor.matmul(out=pg[:], lhsT=w2tt[:], rhs=rt[:],
                                                     start=True, stop=True)
                                    sb = ep.tile([OUT, ST], FP, tag="g2sb")
                                    nc.scalar.activation(out=sb[:], in_=pg[:],
                                                         func=Act.Identity,
                                                         bias=b2ct[:])
                                    stage = ep.tile([P, 4, OUT], F16, tag="stg")
                                else:
                                    sb = ep.tile([OUT, ST], FP, tag="zsb")
                                    nc.scalar.activation(out=sb[:], in_=pst[:],
                                                         func=Act.Copy)
                                    stage = ep.tile([P, 4, OUT], FP, tag="stgz")
                                for j in range(4):
                                    tp = ptp.tile([P, OUT], FP, space="PSUM", tag="tp")
                                    nc.tensor.transpose(
                                        out=tp[:], in_=sb[:, j * P:(j + 1) * P],
                                        identity=id64[:])
                                    nc.scalar.activation(out=stage[:, j, :], in_=tp[:],
                                                         func=Act.Copy)
                                if do_g2:
                                    nc.sync.dma_start(
                                        out=out_local[st][:, :, 0:OUT].transpose([1, 0, 2]),
                                        in_=stage[:])
                                else:
                                    nc.sync.dma_start(
                                        out=out_local[st].transpose([1, 0, 2]),
                                        in_=stage[:])

                if upto != 'A':
                    agg_layer(1, G1, HID, g2_local, do_g2=True)
                    with nc.named_scope("ag2"):
                        _collective(g2_local[:].opt(), G2, SH)
                if upto in ('L2', 'full'):
                    agg_layer(2, G2, OUT, z_local, do_g2=False)
                    with nc.named_scope("ag3"):
                        _collective(z_local[:].opt(), Z, SH)

                # ================= Phase D: decode =================
                if upto == 'full':
                 with nc.named_scope("decode"):
                    with (tc.tile_pool(name="qs", bufs=1) as qs,
                          tc.tile_pool(name="qd", bufs=2) as qd,
                          tc.tile_pool(name="qo", bufs=1) as qo):
                        red_all = qo.tile([P, NQCH], FP, tag="redall")

                        def gath(pool, tag, seg, cb, nch, idxt):
                            t = pool.tile([P, nch, OUT], FP, tag=tag)
                            for g0 in range(0, nch, GMAX):
                                n = min(GMAX, nch - g0)
                                nc.gpsimd.dma_gather(
                                    out_ap=t[:, g0:g0 + n, :],
                                    in_ap=Z[seg * SEG_R:(seg + 1) * SEG_R, :],
                                    idxs_ap=idxt[:, (cb + g0) * P // 16:
                                                 (cb + g0 + n) * P // 16],
                                    num_idxs=n * P, num_idxs_reg=n * P,
                                    elem_size=OUT, queue_num=_next_q())
                            return t

                        for (cb, nch, ss) in src_runs:
                            zs = gath(qs, "zs", ss, cb, nch, qst)
                            for (cb2, nch2, ds) in [x for x in dst_runs
                                                    if cb <= x[0] < cb + nch]:
                                zd = gath(qd, "zd", ds, cb2, nch2, qdt)
                                prod = qd.tile([P, nch2, OUT], FP, tag="prod")
                                nc.vector.tensor_tensor(
                                    out=prod[:], in0=zs[:, cb2 - cb:cb2 - cb + nch2, :],
                                    in1=zd[:], op=mybir.AluOpType.mult)
                                nc.vector.tensor_reduce(
                                    out=red_all[:, cb2:cb2 + nch2], in_=prod[:],
                                    axis=mybir.AxisListType.X, op=mybir.AluOpType.add)
                        nc.sync.dma_start(out=o_dec[:], in_=red_all[:])

    nc.compile()
    return nc


_BUILD_CACHE = {}


def _meta_key(meta):
    import hashlib
    h = hashlib.sha256()
    h.update(np.asarray(meta["chunk_win"]).tobytes())
    h.update(np.asarray(meta["chunk_seg"]).tobytes())
    h.update(np.asarray(meta["st_seg_nchunks"]).tobytes())
    h.update(repr((meta["NCH"], meta["NQCH"], meta["src_runs"], meta["dst_runs"])).encode())
    return h.hexdigest()


def kernel(**inputs):
    in_maps, meta = _preprocess(inputs)
    key = _meta_key(meta)
    if key not in _BUILD_CACHE:
        _BUILD_CACHE[key] = _build(meta)
    nc = _BUILD_CACHE[key]
    res = run_bass_kernel_spmd(nc, in_maps, core_ids=list(range(NCORES)))
    kernel.last_results = res

    out = np.zeros(EQ, dtype=np.float32)
    perm_dec = meta["perm_dec"]
    for r in range(NCORES):
        od = res.results[r]["out"]
        pr = perm_dec[r]
        valid = pr >= 0
        out[r * EQC + pr[valid]] = od.T[valid]
    return out

